# revision 52
# baseline (speedup 1.0000x reference)
"""Multi-head attention forward (b=8, n=2048, dim=512, heads=8, dh=64) on 8
Trainium2 NeuronCores.

Sharding: data-parallel over batch — core i computes the full attention layer
for batch element i (weights replicated, no collectives).

Per-core pipeline (final: quadrant-tiled sims, exp-paced steady state):
  1. x [2048,512] -> bf16 -> PE-transpose -> xT [512,2048] (transposes ride
     a bf16 view of the shared fp32 PSUM ring)
  2. qkT = w_qk.T @ xT [1024,2048] with the q-feature columns of w_qkv
     PRE-SCALED by dh^-0.5*log2(e) so sim arrives in log2 units;
     v = x @ w_v [2048,512] (+ ones col per head for softmax rowsums)
  3. attention in 16 blocks (head pair pr x 512-query chunk qc):
     per key tile j: FOUR 64-feature x 64-key sim matmuls in disjoint
     128x128-array quadrants (tile_position (64h, 64kh), output partition
     base = col group) — pairs stream truly concurrently, unlike
     full-width row-group pairs whose PSUM drains serialize; fp32 PSUM
     tiles [128,2,512] (2 banks, triple-buffered shared pool = 6 banks).
     exp on ScalarE (scale=ln2) directly from PSUM — measured faster than
     any VectorE-assisted scheme: DVE's PSUM port is ~4B/cyc/lane so
     staging 33M elements through SBUF loses more than ScalarE/PE PSUM
     contention costs. attnv accumulates [65,512] per head (row 64 =
     rowsum) in 2 PSUM banks, lagging PENDING_DEPTH=3 key tiles behind
     exp so the PE never stalls on a single ex tile.
     Block end: unnormalized out + den rows leave PSUM via 4 independent
     VectorE copies (ScalarE must not touch PSUM: catastrophic PE
     interference, measured 2.3x degradation).
  4. per head pair: one reciprocal_approx_fast over both heads' dens,
     gpsimd partition_broadcast, then in-place normalization multiplies
     (gpsimd + vector split) on the bf16 attn_outT slices.
  5. out = attn_outT.T @ w_out after the last pair normalizes (the
     out-proj matmuls share the sim PSUM pool).
  qk tiles for later head pairs are emitted between early blocks to fill
  TensorE slack under the exp-paced steady state.

A validated-but-unused fast path remains behind EXP_MODE flags: a custom
DVE op (EXP2_CORR_ANT) computing corrected Schraudolph exp2 from int32
bits at ~0.8% max rel err (bit-exact pieces verified on HW); per-j
VectorE work composes badly with the in-order engine queues, so the
all-ScalarE config wins end to end.
"""

import numpy as np

import concourse.bass as bass
import concourse.mybir as mybir
import concourse.tile as tile
from concourse import bacc
from concourse import dve_ops
from concourse.dve_spec import Spec, Src0, C0, C1, C2, One, Bin, lower
from concourse.dve_uop import AluOp, DveOpSpec
from concourse.masks import make_identity

FP32 = mybir.dt.float32
BF16 = mybir.dt.bfloat16
FP16 = mybir.dt.float16
I32 = mybir.dt.int32

B = 8
N = 2048
D = 512
H = 8
DH = 64
F3 = 3 * D
SCALE = DH**-0.5
LOG2E = 1.4426950408889634
LN2 = 0.6931471805599453
A0 = SCALE * LOG2E  # q-feature pre-scale: sim arrives in log2 units
P = 128
NT = N // P  # 16 token tiles
CT = D // P  # 4 contraction tiles over dim

# Schraudolph-with-correction constants (validated on HW, micro_dve*.py)
MASK_F = float(np.int32(0x007FFFFF).view(np.float32))
C_CORR = 0.2354900174785089  # f = 1 + C*(v-1)(2-v) = 1 + (+C)(v-1)(v-2)*(-1)
BITS_A = 8388608.0  # 2^23
BITS_B = 1065353216.0  # 127 << 23

# which key tiles the Vector engine exps (rest go to ScalarE)
DVE_JS = ()
# emit qk/out-proj matmuls between attention blocks (else serial phases)
INTERLEAVE = True
# "sbuf"  : DVE copies sim PSUM->SBUF fp16, ACT exps from SBUF (fast path —
#           ScalarE must NEVER read PSUM while the PE is streaming)
# "hybrid": tiles in SBUF_JS go DVE-copy(fp32)->ACT-sbuf-exp; rest ACT-psum
# "split" | "act" | "dve" | "none" (none = skip exp, for PE-path timing only)
EXP_MODE = "act"
SBUF_JS = (1, 3, 5, 7, 9, 11, 13, 15)
# how many j iterations attnv lags behind exp emission
PENDING_DEPTH = 3
# debug: skip sim matmuls ("nosim") or attnv matmuls ("noattnv")
SKIP = ""
# sim matmuls as 4 quadrant-tiled 64x64-key MMs (2 truly-concurrent pairs)
SIM_QUAD = True
# attnv as key-half pairs (A keys 0-63 + B keys 64-127 etc.)
ATTNV_PAIR = False
# engine for the 8 normalization multiplies: how many of 8 go to gpsimd
NORM_GPSIMD = 4
# put the unnA evacuation copy on ScalarE (frees VectorE, risks PSUM poison)
SCALAR_EVAC = False


def _ref_exp2corr(in0, in1, s0, s1, imm2):
    b = in0.view(np.int32)
    mask = np.float32(s0).view(np.int32)
    oneb = np.float32(s1).view(np.int32)
    v = ((b & mask) | oneb).view(np.float32)
    return (in0 * (1.0 + imm2 * (v - 1.0) * (v - 2.0))).astype(np.float32)


def _register_exp2_op():
    name = "EXP2_CORR_ANT"
    if name in dve_ops._SUB_OPCODE_FOR_NAME:
        return next(op for op in dve_ops.OPS if op.name == name)
    _and = Bin(AluOp.BITWISE_AND, Src0, C0)
    _v = Bin(AluOp.BITWISE_OR, _and, C1)
    _p = _v - One
    _q = _p - One
    _m = _p * _q
    _f = One + _m * C2
    spec = Spec(body=Src0 * _f, reference=_ref_exp2corr)
    shas = {}
    for ver in ("v3", "v4"):
        s = DveOpSpec(name=name, opcode=0, uops=lower(spec, ver=ver), rd1_en=False)
        shas[ver] = s.sha(ver)
    op = dve_ops.DveOp(name, spec, subdim=False, uops_sha=shas)
    row = max(dve_ops._SUB_OPCODE_FOR_NAME.values()) + 1
    assert row < 0x20
    dve_ops.OPS.append(op)
    dve_ops._SUB_OPCODE_FOR_NAME[name] = row
    dve_ops.CUSTOM_DVE_SPECS[name] = spec
    return op


EXP2_OP = _register_exp2_op()


def _attention_body(tc: "tile.TileContext", repeat: int = 1):
    nc = tc.nc
    x = nc.dram_tensor("x", [N, D], FP32, kind="ExternalInput").ap()
    w_qkv = nc.dram_tensor("w_qkv", [D, F3], FP32, kind="ExternalInput").ap()
    w_out = nc.dram_tensor("w_out", [D, D], FP32, kind="ExternalInput").ap()
    out = nc.dram_tensor("out", [N, D], FP32, kind="ExternalOutput").ap()

    with (
        tc.tile_pool(name="const", bufs=1) as const,
        tc.tile_pool(name="persist", bufs=1) as persist,
        tc.tile_pool(name="wstage", bufs=2) as wstage,
    ):
        identity = const.tile([P, P], BF16)
        make_identity(nc, identity)
        ones32 = const.tile([P, 1], FP32)
        nc.vector.memset(ones32, 1.0)

        # q and k features transposed: rows = 1024 q/k features in 8 tiles
        qkT = persist.tile([P, 8, N], BF16)
        # v with tokens on partitions; per head 64 value cols + 1 ones col
        v_aug = persist.tile([P, NT, H * 65], BF16)
        nc.vector.tensor_copy(
            out=v_aug.rearrange("p j (h c) -> p j h c", c=65)[:, :, :, 64:65],
            in_=ones32.to_broadcast([P, NT, H, 1]),
        )
        # attention output (normalized in step 4), heads stacked in pairs:
        # tile t holds heads (2t, 2t+1) at rows 0-63 / 64-127
        attn_outT = persist.tile([P, CT, N], BF16)
        wout_sb = persist.tile([P, CT, D], BF16)
        for t in range(CT):
            ws = wstage.tile([P, F3], FP32, tag="ws")
            nc.sync.dma_start(out=ws[:, :D], in_=w_out[t * P : (t + 1) * P, :])
            nc.vector.tensor_copy(out=wout_sb[:, t, :], in_=ws[:, :D])
        # w_qkv cast to bf16 with q-feature output columns (0:512) pre-scaled
        wqkv_sb = persist.tile([P, CT, F3], BF16)
        for t in range(CT):
            ws = wstage.tile([P, F3], FP32, tag="ws")
            nc.sync.dma_start(out=ws, in_=w_qkv[t * P : (t + 1) * P, :])
            nc.vector.tensor_scalar_mul(
                out=wqkv_sb[:, t, 0:D], in0=ws[:, 0:D], scalar1=float(A0)
            )
            nc.vector.tensor_copy(out=wqkv_sb[:, t, D:F3], in_=ws[:, D:F3])

        consts = (identity, ones32, qkT, v_aug, attn_outT, wout_sb, wqkv_sb)
        for _ in range(repeat):
            _attention_once(tc, x, out, consts)


def _attention_once(tc: "tile.TileContext", x, out, consts):
    nc = tc.nc
    exp_f = mybir.ActivationFunctionType.Exp
    identity, ones32, qkT, v_aug, attn_outT, wout_sb, wqkv_sb = consts

    with (
        tc.tile_pool(name="proj", bufs=1) as proj_pool,
        tc.tile_pool(name="xstage", bufs=4) as xstage,
        tc.tile_pool(name="xbfst", bufs=4) as xbfst,
        # shared PSUM pool: sim tiles + qk/v/out-proj matmul tiles (2 banks
        # each, 3 bufs = 6 banks; pso pool below takes the last 2)
        tc.tile_pool(name="psmm", bufs=3, space="PSUM") as psmm,
        tc.tile_pool(name="pso", bufs=1, space="PSUM") as psop,
        tc.tile_pool(name="ex", bufs=PENDING_DEPTH + 2) as expp,
        tc.tile_pool(name="simsb", bufs=3) as simsbp,
        tc.tile_pool(name="bits", bufs=2) as bitsp,
        tc.tile_pool(name="dpair", bufs=2) as dpairp,
        tc.tile_pool(name="rinv", bufs=1) as rinvp,
        tc.tile_pool(name="r0", bufs=1) as r0p,
        tc.tile_pool(name="db", bufs=1) as dbp,
        tc.tile_pool(name="outstage", bufs=2) as outstage,
    ):
        # ---- load x, cast bf16, transpose to xT [512, 2048] ----
        xT = proj_pool.tile([P, CT, N], BF16)
        for j in range(NT):
            xs = xstage.tile([P, D], FP32)
            nc.sync.dma_start(out=xs, in_=x[j * P : (j + 1) * P, :])
            xbf = xbfst.tile([P, D], BF16)
            nc.vector.tensor_copy(out=xbf, in_=xs)
            # transpose via shared psum ring (bf16 view of an fp32 tile)
            ps = psmm.tile([P, 2, 512], FP32, tag="mm")
            psb = ps.bitcast(BF16).rearrange("p a b -> p (a b)")
            for t in range(CT):
                nc.tensor.transpose(
                    psb[:, t * P : (t + 1) * P], xbf[:, t * P : (t + 1) * P], identity
                )
            nc.vector.tensor_copy(
                out=xT[:, :, j * P : (j + 1) * P],
                in_=psb[:, 0:512].rearrange("p (t q) -> p t q", q=P),
            )

        # ---- projections (qk feature tiles m; v tiles) ----
        def qk_tile(m, on_act=False):
            for nbp in range(2):
                ps = psmm.tile([P, 2, 512], FP32, tag="mm")
                for c in range(CT):
                    for q in range(2):
                        nc.tensor.matmul(
                            ps[:, q, :],
                            wqkv_sb[:, c, m * P : (m + 1) * P],
                            xT[:, c, (nbp * 2 + q) * 512 : (nbp * 2 + q + 1) * 512],
                            start=(c == 0),
                            stop=(c == CT - 1),
                        )
                src = ps.rearrange("p a b -> p (a b)")
                dst = qkT[:, m, nbp * 1024 : (nbp + 1) * 1024]
                if on_act:
                    nc.scalar.copy(out=dst, in_=src)
                else:
                    nc.vector.tensor_copy(out=dst, in_=src)

        def v_tile(jp):
            ps = psmm.tile([P, 2, 512], FP32, tag="mm")
            for c in range(CT):
                for q in range(2):
                    j = jp * 2 + q
                    nc.tensor.matmul(
                        ps[:, q, :],
                        xT[:, c, j * P : (j + 1) * P],
                        wqkv_sb[:, c, 2 * D : 3 * D],
                        start=(c == 0),
                        stop=(c == CT - 1),
                    )
            dst = v_aug[:, jp * 2 : jp * 2 + 2, :].rearrange(
                "p j (h c) -> p j h c", c=65
            )[:, :, :, 0:64]
            srcv = ps.rearrange("p q (h c) -> p q h c", c=64)
            nc.vector.tensor_copy(out=dst, in_=srcv)

        for jp in range(8):
            v_tile(jp)
        qk_tile(0)
        qk_tile(4)

        # ---- out-projection per token-pair tile (interleaved) ----
        def out_proj(jp):
            ps = psmm.tile([P, 2, 512], FP32, tag="mm")
            for t in range(CT):
                for q in range(2):
                    j = jp * 2 + q
                    nc.tensor.matmul(
                        ps[:, q, :],
                        attn_outT[:, t, j * P : (j + 1) * P],
                        wout_sb[:, t, :],
                        start=(t == 0),
                        stop=(t == CT - 1),
                    )
            os_ = outstage.tile([P, 2, D], FP32)
            nc.vector.tensor_copy(out=os_, in_=ps)
            nc.sync.dma_start(
                out=out[jp * 256 : (jp + 1) * 256, :].rearrange(
                    "(q p) d -> p q d", p=P
                ),
                in_=os_,
            )

        # ---- per-pair normalization over dpair[:, lo:hi] ----
        # pr 0..2 normalize their full 2048 q after the 4th block; pr 3
        # normalizes per 512-q chunk so out-proj can start immediately.
        def normalize(pr, dpair, lo, hi, sfx=""):
            w = hi - lo
            rv = rinvp.tile([33, w], FP32, tag="rv" + sfx)
            nc.vector.reciprocal_approx_fast(out=rv, in_=dpair[:, lo:hi])
            r0 = r0p.tile([1, w], FP32, tag="r0" + sfx)
            nc.vector.tensor_copy(out=r0, in_=rv[32:33, :])
            dbA = dbp.tile([P, w], FP32, tag="bA" + sfx)
            nc.gpsimd.partition_broadcast(dbA, rv[0:1, :])
            dbB = dbp.tile([P, w], FP32, tag="bB" + sfx)
            nc.gpsimd.partition_broadcast(dbB, r0)
            # in-place normalize attn_outT rows; split chunks across engines
            nchunks = max(1, w // 1024)
            csz = w // nchunks
            with nc.allow_low_precision("bf16 attn out"):
                for h, db, rowbase in ((0, dbA, 0), (1, dbB, 64)):
                    for c in range(nchunks):
                        sl = slice(lo + c * csz, lo + (c + 1) * csz)
                        dsl = slice(c * csz, (c + 1) * csz)
                        eng = (
                            nc.gpsimd
                            if (h * nchunks + c) % 4 < NORM_GPSIMD // 2
                            else nc.vector
                        )
                        eng.tensor_mul(
                            out=attn_outT[rowbase : rowbase + 64, pr, sl],
                            in0=attn_outT[rowbase : rowbase + 64, pr, sl],
                            in1=db[rowbase : rowbase + 64, dsl],
                        )

        # ---- attention blocks ----
        if INTERLEAVE:
            qkv_ = lambda m: qk_tile(m, on_act=False)
            extra = {
                0: [lambda: qkv_(1)],
                1: [lambda: qkv_(5)],
                4: [lambda: qkv_(2)],
                5: [lambda: qkv_(6)],
                8: [lambda: qkv_(3)],
                9: [lambda: qkv_(7)],
                # out-proj waits for normalize(pr=3) after block 15; a
                # per-qc pr=3 normalization + earlier out-proj interleave
                # measured 45us WORSE (norm ops gate out-proj, PE stalls)
                15: [lambda jp=jp: out_proj(jp) for jp in range(8)],
            }
        else:
            for m in (1, 5, 2, 6, 3, 7):
                qk_tile(m)
            extra = {15: [lambda jp=jp: out_proj(jp) for jp in range(8)]}

        ex_const = None
        if EXP_MODE == "none":
            ex_const = proj_pool.tile([P, 2, 512], BF16)
            nc.vector.memset(ex_const, 0.001)

        for pr in range(H // 2):
            hA, hB = 2 * pr, 2 * pr + 1
            qt, kt = pr, 4 + pr
            # unnormalized denominators for this pair: row 0 = A, row 32 = B
            dpair = dpairp.tile([33, N], FP32, tag="d")
            for qc in range(4):
                bi = pr * 4 + qc
                qsl = slice(qc * 512, (qc + 1) * 512)
                psoA = psop.tile([65, 512], FP32, tag="A")
                psoB = psop.tile([65, 512], FP32, tag="B")

                def attnv(ex_t, pj):
                    if ATTNV_PAIR and SKIP != "noattnv":
                        first, last = pj == 0, pj == NT - 1
                        # row-tiled pairs: (A keys 0-63 | B keys 64-127),
                        # then (B keys 0-63 | A keys 64-127)
                        for pso, hh, kh, idx in (
                            (psoA, hA, 0, 0),
                            (psoB, hB, 1, 0),
                            (psoB, hB, 0, 1),
                            (psoA, hA, 1, 1),
                        ):
                            rows = slice(64 * kh, 64 * kh + 64)
                            hcol = 0 if pso is psoA else 1
                            nc.tensor.matmul(
                                pso,
                                v_aug[rows, pj, hh * 65 : (hh + 1) * 65],
                                ex_t[rows, hcol, :],
                                start=(first and idx == 0),
                                stop=(last and idx == 1),
                                tile_position=(64 * kh, 0),
                            )
                        return
                    if SKIP == "noattnv":
                        if pj == NT - 1:
                            nc.tensor.matmul(
                                psoA, v_aug[:, 0, 0:65], ex_t[:, 0, :],
                                start=True, stop=True,
                            )
                            nc.tensor.matmul(
                                psoB, v_aug[:, 0, 0:65], ex_t[:, 1, :],
                                start=True, stop=True,
                            )
                        return
                    nc.tensor.matmul(
                        psoA,
                        v_aug[:, pj, hA * 65 : (hA + 1) * 65],
                        ex_t[:, 0, :],
                        start=(pj == 0),
                        stop=(pj == NT - 1),
                    )
                    nc.tensor.matmul(
                        psoB,
                        v_aug[:, pj, hB * 65 : (hB + 1) * 65],
                        ex_t[:, 1, :],
                        start=(pj == 0),
                        stop=(pj == NT - 1),
                    )

                pending = []
                for j in range(NT):
                    simt = None
                    if SKIP != "nosim":
                        simt = psmm.tile([P, 2, 512], FP32, tag="mm")
                        if SIM_QUAD:
                            # four 64-feat x 64-key MMs in disjoint array
                            # quadrants; pairs stream concurrently.
                            # (head h on array rows 64h.., key half kh on
                            #  array cols 64kh -> psum partitions 64kh..)
                            for h, kh in ((0, 0), (1, 1), (0, 1), (1, 0)):
                                rows = slice(64 * h, 64 * h + 64)
                                ksl = slice(
                                    j * P + 64 * kh, j * P + 64 * kh + 64
                                )
                                nc.tensor.matmul(
                                    simt[64 * kh : 64 * kh + 64, h, :],
                                    qkT[rows, kt, ksl],
                                    qkT[rows, qt, qsl],
                                    start=True,
                                    stop=True,
                                    tile_position=(64 * h, 64 * kh),
                                )
                        else:
                            nc.tensor.matmul(
                                simt[:, 0, :],
                                qkT[0:64, kt, j * P : (j + 1) * P],
                                qkT[0:64, qt, qsl],
                                start=True,
                                stop=True,
                                tile_position=(0, 0),
                            )
                            nc.tensor.matmul(
                                simt[:, 1, :],
                                qkT[64:128, kt, j * P : (j + 1) * P],
                                qkT[64:128, qt, qsl],
                                start=True,
                                stop=True,
                                tile_position=(64, 0),
                            )
                    elif EXP_MODE != "none":
                        simt = psmm.tile([P, 2, 512], FP32, tag="mm")
                        nc.vector.memset(simt, 0.5)
                    use_dve = (EXP_MODE == "split" and j in DVE_JS) or (
                        EXP_MODE == "dve"
                    )
                    if EXP_MODE == "none":
                        ex_t = ex_const
                    elif EXP_MODE in ("sbuf", "hybrid"):
                        ex_t = expp.tile([P, 2, 512], BF16, tag="ex")
                        if EXP_MODE == "sbuf" or j in SBUF_JS:
                            sim_sb = simsbp.tile([P, 2, 512], FP32, tag="ss")
                            nc.vector.tensor_copy(out=sim_sb, in_=simt)
                            src_ap = sim_sb
                        else:
                            src_ap = simt
                        nc.scalar.activation(
                            out=ex_t.rearrange("p a b -> p (a b)"),
                            in_=src_ap.rearrange("p a b -> p (a b)"),
                            func=exp_f,
                            scale=LN2,
                        )
                    elif use_dve:
                        ex_t = expp.tile([P, 2, 512], BF16, tag="ex")
                        bits = bitsp.tile([P, 2, 512], I32, tag="bits")
                        nc.vector.tensor_scalar(
                            out=bits,
                            in0=simt,
                            scalar1=BITS_A,
                            scalar2=BITS_B,
                            op0=mybir.AluOpType.mult,
                            op1=mybir.AluOpType.add,
                        )
                        nc.vector._custom_dve(
                            EXP2_OP,
                            out=ex_t,
                            in0=bits.bitcast(FP32),
                            s0=MASK_F,
                            s1=1.0,
                            imm2=C_CORR,
                        )
                    else:
                        ex_t = expp.tile([P, 2, 512], BF16, tag="ex")
                        nc.scalar.activation(
                            out=ex_t.rearrange("p a b -> p (a b)"),
                            in_=simt.rearrange("p a b -> p (a b)"),
                            func=exp_f,
                            scale=LN2,
                        )
                    pending.append((ex_t, j))
                    if len(pending) > PENDING_DEPTH:
                        attnv(*pending.pop(0))
                for p_ in pending:
                    attnv(*p_)

                # evacuate: 4 independent copies, pso banks free ASAP
                # (VectorE only — ScalarE reading PSUM stalls the PE)
                with nc.allow_low_precision("bf16 attn out"):
                    if SCALAR_EVAC:
                        nc.scalar.copy(
                            out=attn_outT[0:64, pr, qsl], in_=psoA[0:64, :]
                        )
                    else:
                        nc.vector.tensor_copy(
                            out=attn_outT[0:64, pr, qsl], in_=psoA[0:64, :]
                        )
                    nc.vector.tensor_copy(
                        out=attn_outT[64:128, pr, qsl], in_=psoB[0:64, :]
                    )
                nc.vector.tensor_copy(
                    out=dpair[0:1, qsl], in_=psoA[64:65, :]
                )
                nc.vector.tensor_copy(
                    out=dpair[32:33, qsl], in_=psoB[64:65, :]
                )

                if EXP_MODE != "none" and bi % 4 == 3:
                    normalize(pr, dpair, 0, N)
                for thunk in extra.get(bi, ()):
                    thunk()


_CACHE: dict = {}


def build_nc(repeat: int = 1) -> "bass.Bass":
    key = (
        "nc", repeat, EXP_MODE, DVE_JS, INTERLEAVE, NORM_GPSIMD,
        PENDING_DEPTH, SKIP, SBUF_JS, SIM_QUAD, ATTNV_PAIR, SCALAR_EVAC,
    )
    if key not in _CACHE:
        nc = bacc.Bacc("TRN2", target_bir_lowering=False, debug=False)
        with tile.TileContext(nc) as tc:
            _attention_body(tc, repeat=repeat)
        nc.compile()
        _CACHE[key] = nc
    return _CACHE[key]


def kernel(x: np.ndarray, w_qkv: np.ndarray, w_out: np.ndarray) -> np.ndarray:
    from concourse.bass_utils import run_bass_kernel_spmd

    nc = build_nc()
    x = np.ascontiguousarray(np.asarray(x, dtype=np.float32))
    w_qkv = np.ascontiguousarray(np.asarray(w_qkv, dtype=np.float32))
    w_out = np.ascontiguousarray(np.asarray(w_out, dtype=np.float32))
    in_maps = [
        {"x": x[i], "w_qkv": w_qkv, "w_out": w_out} for i in range(B)
    ]
    res = run_bass_kernel_spmd(nc, in_maps, core_ids=list(range(B)))
    return np.stack([r["out"] for r in res.results], axis=0)


# revision 60
# speedup vs baseline: 1.0159x; 1.0159x over previous
"""Multi-head attention forward (b=8, n=2048, dim=512, heads=8, dh=64) on 8
Trainium2 NeuronCores.

Sharding: data-parallel over batch — core i computes the full attention layer
for batch element i (weights replicated, no collectives).

Per-core pipeline (final: quadrant-tiled sims, exp-paced steady state):
  1. x [2048,512] -> bf16 -> PE-transpose -> xT [512,2048] (transposes ride
     a bf16 view of the shared fp32 PSUM ring)
  2. qkT = w_qk.T @ xT [1024,2048] with the q-feature columns of w_qkv
     PRE-SCALED by dh^-0.5*log2(e) so sim arrives in log2 units;
     v = x @ w_v [2048,512] (+ ones col per head for softmax rowsums)
  3. attention in 16 blocks (head pair pr x 512-query chunk qc):
     per key tile j: FOUR 64-feature x 64-key sim matmuls in disjoint
     128x128-array quadrants (tile_position (64h, 64kh), output partition
     base = col group) — pairs stream truly concurrently, unlike
     full-width row-group pairs whose PSUM drains serialize; fp32 PSUM
     tiles [128,2,512] (2 banks, triple-buffered shared pool = 6 banks).
     exp on ScalarE (scale=ln2) directly from PSUM — measured faster than
     any VectorE-assisted scheme: DVE's PSUM port is ~4B/cyc/lane so
     staging 33M elements through SBUF loses more than ScalarE/PE PSUM
     contention costs. attnv accumulates [65,512] per head (row 64 =
     rowsum) in 2 PSUM banks, lagging PENDING_DEPTH=3 key tiles behind
     exp so the PE never stalls on a single ex tile.
     Block end: unnormalized out + den rows leave PSUM via 4 independent
     VectorE copies (ScalarE must not touch PSUM: catastrophic PE
     interference, measured 2.3x degradation).
  4. per head pair: one reciprocal_approx_fast over both heads' dens,
     gpsimd partition_broadcast, then in-place normalization multiplies
     (gpsimd + vector split) on the bf16 attn_outT slices.
  5. out = attn_outT.T @ w_out after the last pair normalizes (the
     out-proj matmuls share the sim PSUM pool).
  qk tiles for later head pairs are emitted between early blocks to fill
  TensorE slack under the exp-paced steady state.

A validated-but-unused fast path remains behind EXP_MODE flags: a custom
DVE op (EXP2_CORR_ANT) computing corrected Schraudolph exp2 from int32
bits at ~0.8% max rel err (bit-exact pieces verified on HW); per-j
VectorE work composes badly with the in-order engine queues, so the
all-ScalarE config wins end to end.
"""

import numpy as np

import concourse.bass as bass
import concourse.mybir as mybir
import concourse.tile as tile
from concourse import bacc
from concourse import dve_ops
from concourse.dve_spec import Spec, Src0, C0, C1, C2, One, Bin, lower
from concourse.dve_uop import AluOp, DveOpSpec
from concourse.masks import make_identity

FP32 = mybir.dt.float32
BF16 = mybir.dt.bfloat16
FP16 = mybir.dt.float16
I32 = mybir.dt.int32

B = 8
N = 2048
D = 512
H = 8
DH = 64
F3 = 3 * D
SCALE = DH**-0.5
LOG2E = 1.4426950408889634
LN2 = 0.6931471805599453
A0 = SCALE * LOG2E  # q-feature pre-scale: sim arrives in log2 units
P = 128
NT = N // P  # 16 token tiles
CT = D // P  # 4 contraction tiles over dim

# Schraudolph-with-correction constants (validated on HW, micro_dve*.py)
MASK_F = float(np.int32(0x007FFFFF).view(np.float32))
C_CORR = 0.2354900174785089  # f = 1 + C*(v-1)(2-v) = 1 + (+C)(v-1)(v-2)*(-1)
BITS_A = 8388608.0  # 2^23
BITS_B = 1065353216.0  # 127 << 23

# which key tiles the Vector engine exps (rest go to ScalarE)
DVE_JS = ()
# emit qk/out-proj matmuls between attention blocks (else serial phases)
INTERLEAVE = True
# "sbuf"  : DVE copies sim PSUM->SBUF fp16, ACT exps from SBUF (fast path —
#           ScalarE must NEVER read PSUM while the PE is streaming)
# "hybrid": tiles in SBUF_JS go DVE-copy(fp32)->ACT-sbuf-exp; rest ACT-psum
# "split" | "act" | "dve" | "none" (none = skip exp, for PE-path timing only)
EXP_MODE = "act"
SBUF_JS = (1, 3, 5, 7, 9, 11, 13, 15)
# how many j iterations attnv lags behind exp emission
PENDING_DEPTH = 3
# debug: skip sim matmuls ("nosim") or attnv matmuls ("noattnv")
SKIP = ""
# sim matmuls as 4 quadrant-tiled 64x64-key MMs (2 truly-concurrent pairs)
SIM_QUAD = True
# attnv as key-half pairs (A keys 0-63 + B keys 64-127 etc.)
ATTNV_PAIR = False
# engine for the 8 normalization multiplies: how many of 8 go to gpsimd
NORM_GPSIMD = 4
# exp key tiles in (pair, single) cadence: pairs land in a dedicated 4-bank
# PSUM tile so ScalarE reads 2048-wide (0.98 ns/elem vs 1.31 at 1024)
EXP_PAIRS = False  # pair cadence measured 430us vs 392-403 checkpoint (phase-1 bufs=1 serialization)
PAIR_STARTS = (0, 3, 6, 9, 12)  # j values starting a (j, j+1) pair
# put the unnA evacuation copy on ScalarE (frees VectorE, risks PSUM poison)
SCALAR_EVAC = False


def _ref_exp2corr(in0, in1, s0, s1, imm2):
    b = in0.view(np.int32)
    mask = np.float32(s0).view(np.int32)
    oneb = np.float32(s1).view(np.int32)
    v = ((b & mask) | oneb).view(np.float32)
    return (in0 * (1.0 + imm2 * (v - 1.0) * (v - 2.0))).astype(np.float32)


def _register_exp2_op():
    name = "EXP2_CORR_ANT"
    if name in dve_ops._SUB_OPCODE_FOR_NAME:
        return next(op for op in dve_ops.OPS if op.name == name)
    _and = Bin(AluOp.BITWISE_AND, Src0, C0)
    _v = Bin(AluOp.BITWISE_OR, _and, C1)
    _p = _v - One
    _q = _p - One
    _m = _p * _q
    _f = One + _m * C2
    spec = Spec(body=Src0 * _f, reference=_ref_exp2corr)
    shas = {}
    for ver in ("v3", "v4"):
        s = DveOpSpec(name=name, opcode=0, uops=lower(spec, ver=ver), rd1_en=False)
        shas[ver] = s.sha(ver)
    op = dve_ops.DveOp(name, spec, subdim=False, uops_sha=shas)
    row = max(dve_ops._SUB_OPCODE_FOR_NAME.values()) + 1
    assert row < 0x20
    dve_ops.OPS.append(op)
    dve_ops._SUB_OPCODE_FOR_NAME[name] = row
    dve_ops.CUSTOM_DVE_SPECS[name] = spec
    return op


EXP2_OP = _register_exp2_op()


def _attention_body(tc: "tile.TileContext", repeat: int = 1):
    nc = tc.nc
    x = nc.dram_tensor("x", [N, D], FP32, kind="ExternalInput").ap()
    w_qkv = nc.dram_tensor("w_qkv", [D, F3], FP32, kind="ExternalInput").ap()
    w_out = nc.dram_tensor("w_out", [D, D], FP32, kind="ExternalInput").ap()
    out = nc.dram_tensor("out", [N, D], FP32, kind="ExternalOutput").ap()

    with (
        tc.tile_pool(name="const", bufs=1) as const,
        tc.tile_pool(name="persist", bufs=1) as persist,
        tc.tile_pool(name="wstage", bufs=2) as wstage,
    ):
        identity = const.tile([P, P], BF16)
        make_identity(nc, identity)
        ones32 = const.tile([P, 1], FP32)
        nc.vector.memset(ones32, 1.0)

        # q and k features transposed: rows = 1024 q/k features in 8 tiles
        qkT = persist.tile([P, 8, N], BF16)
        # v with tokens on partitions; per head 64 value cols + 1 ones col
        v_aug = persist.tile([P, NT, H * 65], BF16)
        nc.vector.tensor_copy(
            out=v_aug.rearrange("p j (h c) -> p j h c", c=65)[:, :, :, 64:65],
            in_=ones32.to_broadcast([P, NT, H, 1]),
        )
        # attention output (normalized in step 4), heads stacked in pairs:
        # tile t holds heads (2t, 2t+1) at rows 0-63 / 64-127
        attn_outT = persist.tile([P, CT, N], BF16)
        wout_sb = persist.tile([P, CT, D], BF16)
        for t in range(CT):
            ws = wstage.tile([P, F3], FP32, tag="ws")
            nc.sync.dma_start(out=ws[:, :D], in_=w_out[t * P : (t + 1) * P, :])
            nc.vector.tensor_copy(out=wout_sb[:, t, :], in_=ws[:, :D])
        # w_qkv cast to bf16 with q-feature output columns (0:512) pre-scaled
        wqkv_sb = persist.tile([P, CT, F3], BF16)
        for t in range(CT):
            ws = wstage.tile([P, F3], FP32, tag="ws")
            nc.sync.dma_start(out=ws, in_=w_qkv[t * P : (t + 1) * P, :])
            nc.vector.tensor_scalar_mul(
                out=wqkv_sb[:, t, 0:D], in0=ws[:, 0:D], scalar1=float(A0)
            )
            nc.vector.tensor_copy(out=wqkv_sb[:, t, D:F3], in_=ws[:, D:F3])

        consts = (identity, ones32, qkT, v_aug, attn_outT, wout_sb, wqkv_sb)
        for _ in range(repeat):
            _attention_once(tc, x, out, consts)


def _attention_once(tc: "tile.TileContext", x, out, consts):
    nc = tc.nc
    exp_f = mybir.ActivationFunctionType.Exp
    identity, ones32, qkT, v_aug, attn_outT, wout_sb, wqkv_sb = consts

    with (
        tc.tile_pool(name="proj", bufs=1) as proj_pool,
        tc.tile_pool(name="xstage", bufs=4) as xstage,
        tc.tile_pool(name="xbfst", bufs=4) as xbfst,
        # shared PSUM pool: sim tiles + qk/v/out-proj matmul tiles (2 banks
        # each, 3 bufs = 6 banks; pso pool below takes the last 2)
        tc.tile_pool(
            name="psmm", bufs=1 if EXP_PAIRS else 3, space="PSUM"
        ) as psmm,
        tc.tile_pool(name="pspair", bufs=1, space="PSUM") as pspair,
        tc.tile_pool(name="pso", bufs=1, space="PSUM") as psop,
        tc.tile_pool(name="ex", bufs=PENDING_DEPTH + 2) as expp,
        tc.tile_pool(name="simsb", bufs=3) as simsbp,
        tc.tile_pool(name="bits", bufs=2) as bitsp,
        tc.tile_pool(name="dpair", bufs=2) as dpairp,
        tc.tile_pool(name="rinv", bufs=1) as rinvp,
        tc.tile_pool(name="r0", bufs=1) as r0p,
        tc.tile_pool(name="db", bufs=1) as dbp,
        tc.tile_pool(name="outstage", bufs=2) as outstage,
    ):
        # ---- load x, cast bf16, transpose to xT [512, 2048] ----
        xT = proj_pool.tile([P, CT, N], BF16)
        for j in range(NT):
            xs = xstage.tile([P, D], FP32)
            nc.sync.dma_start(out=xs, in_=x[j * P : (j + 1) * P, :])
            xbf = xbfst.tile([P, D], BF16)
            nc.vector.tensor_copy(out=xbf, in_=xs)
            # transpose via shared psum ring (bf16 view of an fp32 tile)
            ps = psmm.tile([P, 2, 512], FP32, tag="mm")
            psb = ps.bitcast(BF16).rearrange("p a b -> p (a b)")
            for t in range(CT):
                nc.tensor.transpose(
                    psb[:, t * P : (t + 1) * P], xbf[:, t * P : (t + 1) * P], identity
                )
            nc.vector.tensor_copy(
                out=xT[:, :, j * P : (j + 1) * P],
                in_=psb[:, 0:512].rearrange("p (t q) -> p t q", q=P),
            )

        # ---- projections (qk feature tiles m; v tiles) ----
        def qk_tile(m, on_act=False):
            for nbp in range(2):
                ps = psmm.tile([P, 2, 512], FP32, tag="mm")
                for c in range(CT):
                    for q in range(2):
                        nc.tensor.matmul(
                            ps[:, q, :],
                            wqkv_sb[:, c, m * P : (m + 1) * P],
                            xT[:, c, (nbp * 2 + q) * 512 : (nbp * 2 + q + 1) * 512],
                            start=(c == 0),
                            stop=(c == CT - 1),
                        )
                src = ps.rearrange("p a b -> p (a b)")
                dst = qkT[:, m, nbp * 1024 : (nbp + 1) * 1024]
                if on_act:
                    nc.scalar.copy(out=dst, in_=src)
                else:
                    nc.vector.tensor_copy(out=dst, in_=src)

        def v_tile(jp):
            ps = psmm.tile([P, 2, 512], FP32, tag="mm")
            for c in range(CT):
                for q in range(2):
                    j = jp * 2 + q
                    nc.tensor.matmul(
                        ps[:, q, :],
                        xT[:, c, j * P : (j + 1) * P],
                        wqkv_sb[:, c, 2 * D : 3 * D],
                        start=(c == 0),
                        stop=(c == CT - 1),
                    )
            dst = v_aug[:, jp * 2 : jp * 2 + 2, :].rearrange(
                "p j (h c) -> p j h c", c=65
            )[:, :, :, 0:64]
            srcv = ps.rearrange("p q (h c) -> p q h c", c=64)
            nc.vector.tensor_copy(out=dst, in_=srcv)

        for jp in range(8):
            v_tile(jp)
        qk_tile(0)
        qk_tile(4)

        # ---- out-projection per token-pair tile (interleaved) ----
        def out_proj(jp):
            # alternate pools so the tail isn't serialized on one buffer
            if EXP_PAIRS and EXP_MODE == "act" and jp % 2 == 0:
                psbig = pspair.tile([P, 2, 2, 512], FP32, tag="pp")
                ps = psbig[:, 0, :, :]
            else:
                ps = psmm.tile([P, 2, 512], FP32, tag="mm")
            for t in range(CT):
                for q in range(2):
                    j = jp * 2 + q
                    nc.tensor.matmul(
                        ps[:, q, :],
                        attn_outT[:, t, j * P : (j + 1) * P],
                        wout_sb[:, t, :],
                        start=(t == 0),
                        stop=(t == CT - 1),
                    )
            os_ = outstage.tile([P, 2, D], FP32)
            nc.vector.tensor_copy(out=os_, in_=ps)
            nc.sync.dma_start(
                out=out[jp * 256 : (jp + 1) * 256, :].rearrange(
                    "(q p) d -> p q d", p=P
                ),
                in_=os_,
            )

        # ---- per-pair normalization over dpair[:, lo:hi] ----
        # pr 0..2 normalize their full 2048 q after the 4th block; pr 3
        # normalizes per 512-q chunk so out-proj can start immediately.
        def normalize(pr, dpair, lo, hi, sfx=""):
            w = hi - lo
            rv = rinvp.tile([33, w], FP32, tag="rv" + sfx)
            nc.vector.reciprocal_approx_fast(out=rv, in_=dpair[:, lo:hi])
            r0 = r0p.tile([1, w], FP32, tag="r0" + sfx)
            nc.vector.tensor_copy(out=r0, in_=rv[32:33, :])
            dbA = dbp.tile([P, w], FP32, tag="bA" + sfx)
            nc.gpsimd.partition_broadcast(dbA, rv[0:1, :])
            dbB = dbp.tile([P, w], FP32, tag="bB" + sfx)
            nc.gpsimd.partition_broadcast(dbB, r0)
            # in-place normalize attn_outT rows; split chunks across engines
            nchunks = max(1, w // 1024)
            csz = w // nchunks
            with nc.allow_low_precision("bf16 attn out"):
                for h, db, rowbase in ((0, dbA, 0), (1, dbB, 64)):
                    for c in range(nchunks):
                        sl = slice(lo + c * csz, lo + (c + 1) * csz)
                        dsl = slice(c * csz, (c + 1) * csz)
                        eng = (
                            nc.gpsimd
                            if (h * nchunks + c) % 4 < NORM_GPSIMD // 2
                            else nc.vector
                        )
                        eng.tensor_mul(
                            out=attn_outT[rowbase : rowbase + 64, pr, sl],
                            in0=attn_outT[rowbase : rowbase + 64, pr, sl],
                            in1=db[rowbase : rowbase + 64, dsl],
                        )

        # ---- attention blocks ----
        if INTERLEAVE:
            qkv_ = lambda m: qk_tile(m, on_act=False)
            extra = {
                0: [lambda: qkv_(1)],
                1: [lambda: qkv_(5)],
                4: [lambda: qkv_(2)],
                5: [lambda: qkv_(6)],
                8: [lambda: qkv_(3)],
                9: [lambda: qkv_(7)],
                # out-proj waits for normalize(pr=3) after block 15; a
                # per-qc pr=3 normalization + earlier out-proj interleave
                # measured 45us WORSE (norm ops gate out-proj, PE stalls)
                15: [lambda jp=jp: out_proj(jp) for jp in range(8)],
            }
        else:
            for m in (1, 5, 2, 6, 3, 7):
                qk_tile(m)
            extra = {15: [lambda jp=jp: out_proj(jp) for jp in range(8)]}

        ex_const = None
        if EXP_MODE == "none":
            ex_const = proj_pool.tile([P, 2, 512], BF16)
            nc.vector.memset(ex_const, 0.001)

        for pr in range(H // 2):
            hA, hB = 2 * pr, 2 * pr + 1
            qt, kt = pr, 4 + pr
            # unnormalized denominators for this pair: row 0 = A, row 32 = B
            dpair = dpairp.tile([33, N], FP32, tag="d")
            for qc in range(4):
                bi = pr * 4 + qc
                qsl = slice(qc * 512, (qc + 1) * 512)
                psoA = psop.tile([65, 512], FP32, tag="A")
                psoB = psop.tile([65, 512], FP32, tag="B")

                def attnv(ex_t, pj):
                    if ATTNV_PAIR and SKIP != "noattnv":
                        first, last = pj == 0, pj == NT - 1
                        # row-tiled pairs: (A keys 0-63 | B keys 64-127),
                        # then (B keys 0-63 | A keys 64-127)
                        for pso, hh, kh, idx in (
                            (psoA, hA, 0, 0),
                            (psoB, hB, 1, 0),
                            (psoB, hB, 0, 1),
                            (psoA, hA, 1, 1),
                        ):
                            rows = slice(64 * kh, 64 * kh + 64)
                            hcol = 0 if pso is psoA else 1
                            nc.tensor.matmul(
                                pso,
                                v_aug[rows, pj, hh * 65 : (hh + 1) * 65],
                                ex_t[rows, hcol, :],
                                start=(first and idx == 0),
                                stop=(last and idx == 1),
                                tile_position=(64 * kh, 0),
                            )
                        return
                    if SKIP == "noattnv":
                        if pj == NT - 1:
                            nc.tensor.matmul(
                                psoA, v_aug[:, 0, 0:65], ex_t[:, 0, :],
                                start=True, stop=True,
                            )
                            nc.tensor.matmul(
                                psoB, v_aug[:, 0, 0:65], ex_t[:, 1, :],
                                start=True, stop=True,
                            )
                        return
                    nc.tensor.matmul(
                        psoA,
                        v_aug[:, pj, hA * 65 : (hA + 1) * 65],
                        ex_t[:, 0, :],
                        start=(pj == 0),
                        stop=(pj == NT - 1),
                    )
                    nc.tensor.matmul(
                        psoB,
                        v_aug[:, pj, hB * 65 : (hB + 1) * 65],
                        ex_t[:, 1, :],
                        start=(pj == 0),
                        stop=(pj == NT - 1),
                    )

                def sim_mms(j, dst):
                    # four 64-feat x 64-key MMs in disjoint array
                    # quadrants; pairs stream concurrently.
                    # (head h on array rows 64h.., key half kh on
                    #  array cols 64kh -> psum partitions 64kh..)
                    for h, kh in ((0, 0), (1, 1), (0, 1), (1, 0)):
                        rows = slice(64 * h, 64 * h + 64)
                        ksl = slice(j * P + 64 * kh, j * P + 64 * kh + 64)
                        nc.tensor.matmul(
                            dst[64 * kh : 64 * kh + 64, h, :],
                            qkT[rows, kt, ksl],
                            qkT[rows, qt, qsl],
                            start=True,
                            stop=True,
                            tile_position=(64 * h, 64 * kh),
                        )

                use_pairs = (
                    EXP_PAIRS and EXP_MODE == "act" and SKIP == "" and SIM_QUAD
                )
                pending = []
                j = 0
                while j < NT:
                    if use_pairs and j in PAIR_STARTS:
                        # 2048-wide exp: two key tiles in one 4-bank tile
                        pt = pspair.tile([P, 2, 2, 512], FP32, tag="pp")
                        sim_mms(j, pt[:, 0, :, :])
                        sim_mms(j + 1, pt[:, 1, :, :])
                        ex2 = expp.tile([P, 2, 2, 512], BF16, tag="ex2")
                        nc.scalar.activation(
                            out=ex2.rearrange("p a b c -> p (a b c)"),
                            in_=pt.rearrange("p a b c -> p (a b c)"),
                            func=exp_f,
                            scale=LN2,
                        )
                        pending.append((ex2[:, 0, :, :], j))
                        pending.append((ex2[:, 1, :, :], j + 1))
                        while len(pending) > PENDING_DEPTH:
                            attnv(*pending.pop(0))
                        j += 2
                        continue
                    simt = None
                    if SKIP != "nosim":
                        simt = psmm.tile([P, 2, 512], FP32, tag="mm")
                        if SIM_QUAD:
                            sim_mms(j, simt)
                        else:
                            nc.tensor.matmul(
                                simt[:, 0, :],
                                qkT[0:64, kt, j * P : (j + 1) * P],
                                qkT[0:64, qt, qsl],
                                start=True,
                                stop=True,
                                tile_position=(0, 0),
                            )
                            nc.tensor.matmul(
                                simt[:, 1, :],
                                qkT[64:128, kt, j * P : (j + 1) * P],
                                qkT[64:128, qt, qsl],
                                start=True,
                                stop=True,
                                tile_position=(64, 0),
                            )
                    elif EXP_MODE != "none":
                        simt = psmm.tile([P, 2, 512], FP32, tag="mm")
                        nc.vector.memset(simt, 0.5)
                    use_dve = (EXP_MODE == "split" and j in DVE_JS) or (
                        EXP_MODE == "dve"
                    )
                    if EXP_MODE == "none":
                        ex_t = ex_const
                    elif EXP_MODE in ("sbuf", "hybrid"):
                        ex_t = expp.tile([P, 2, 512], BF16, tag="ex")
                        if EXP_MODE == "sbuf" or j in SBUF_JS:
                            sim_sb = simsbp.tile([P, 2, 512], FP32, tag="ss")
                            nc.vector.tensor_copy(out=sim_sb, in_=simt)
                            src_ap = sim_sb
                        else:
                            src_ap = simt
                        nc.scalar.activation(
                            out=ex_t.rearrange("p a b -> p (a b)"),
                            in_=src_ap.rearrange("p a b -> p (a b)"),
                            func=exp_f,
                            scale=LN2,
                        )
                    elif use_dve:
                        ex_t = expp.tile([P, 2, 512], BF16, tag="ex")
                        bits = bitsp.tile([P, 2, 512], I32, tag="bits")
                        nc.vector.tensor_scalar(
                            out=bits,
                            in0=simt,
                            scalar1=BITS_A,
                            scalar2=BITS_B,
                            op0=mybir.AluOpType.mult,
                            op1=mybir.AluOpType.add,
                        )
                        nc.vector._custom_dve(
                            EXP2_OP,
                            out=ex_t,
                            in0=bits.bitcast(FP32),
                            s0=MASK_F,
                            s1=1.0,
                            imm2=C_CORR,
                        )
                    else:
                        ex_t = expp.tile([P, 2, 512], BF16, tag="ex")
                        nc.scalar.activation(
                            out=ex_t.rearrange("p a b -> p (a b)"),
                            in_=simt.rearrange("p a b -> p (a b)"),
                            func=exp_f,
                            scale=LN2,
                        )
                    pending.append((ex_t, j))
                    if len(pending) > PENDING_DEPTH:
                        attnv(*pending.pop(0))
                    j += 1
                for p_ in pending:
                    attnv(*p_)

                # evacuate: 4 independent copies, pso banks free ASAP
                # (VectorE only — ScalarE reading PSUM stalls the PE)
                with nc.allow_low_precision("bf16 attn out"):
                    if SCALAR_EVAC:
                        nc.scalar.copy(
                            out=attn_outT[0:64, pr, qsl], in_=psoA[0:64, :]
                        )
                    else:
                        nc.vector.tensor_copy(
                            out=attn_outT[0:64, pr, qsl], in_=psoA[0:64, :]
                        )
                    nc.vector.tensor_copy(
                        out=attn_outT[64:128, pr, qsl], in_=psoB[0:64, :]
                    )
                nc.vector.tensor_copy(
                    out=dpair[0:1, qsl], in_=psoA[64:65, :]
                )
                nc.vector.tensor_copy(
                    out=dpair[32:33, qsl], in_=psoB[64:65, :]
                )

                if EXP_MODE != "none" and bi % 4 == 3:
                    normalize(pr, dpair, 0, N)
                for thunk in extra.get(bi, ()):
                    thunk()


_CACHE: dict = {}


def build_nc(repeat: int = 1) -> "bass.Bass":
    key = (
        "nc", repeat, EXP_MODE, DVE_JS, INTERLEAVE, NORM_GPSIMD,
        PENDING_DEPTH, SKIP, SBUF_JS, SIM_QUAD, ATTNV_PAIR, SCALAR_EVAC,
        EXP_PAIRS, PAIR_STARTS,
    )
    if key not in _CACHE:
        nc = bacc.Bacc("TRN2", target_bir_lowering=False, debug=False)
        with tile.TileContext(nc) as tc:
            _attention_body(tc, repeat=repeat)
        nc.compile()
        _CACHE[key] = nc
    return _CACHE[key]


def kernel(x: np.ndarray, w_qkv: np.ndarray, w_out: np.ndarray) -> np.ndarray:
    from concourse.bass_utils import run_bass_kernel_spmd

    nc = build_nc()
    x = np.ascontiguousarray(np.asarray(x, dtype=np.float32))
    w_qkv = np.ascontiguousarray(np.asarray(w_qkv, dtype=np.float32))
    w_out = np.ascontiguousarray(np.asarray(w_out, dtype=np.float32))
    in_maps = [
        {"x": x[i], "w_qkv": w_qkv, "w_out": w_out} for i in range(B)
    ]
    res = run_bass_kernel_spmd(nc, in_maps, core_ids=list(range(B)))
    return np.stack([r["out"] for r in res.results], axis=0)


# revision 61
# speedup vs baseline: 1.0182x; 1.0022x over previous
"""Multi-head attention forward (b=8, n=2048, dim=512, heads=8, dh=64) on 8
Trainium2 NeuronCores.

Sharding: data-parallel over batch — core i computes the full attention layer
for batch element i (weights replicated, no collectives).

Per-core pipeline (final: quadrant-tiled sims, exp-paced steady state):
  1. x [2048,512] -> bf16 -> PE-transpose -> xT [512,2048] (transposes ride
     a bf16 view of the shared fp32 PSUM ring)
  2. qkT = w_qk.T @ xT [1024,2048] with the q-feature columns of w_qkv
     PRE-SCALED by dh^-0.5*log2(e) so sim arrives in log2 units;
     v = x @ w_v [2048,512] (+ ones col per head for softmax rowsums)
  3. attention in 16 blocks (head pair pr x 512-query chunk qc):
     per key tile j: FOUR 64-feature x 64-key sim matmuls in disjoint
     128x128-array quadrants (tile_position (64h, 64kh), output partition
     base = col group) — pairs stream truly concurrently, unlike
     full-width row-group pairs whose PSUM drains serialize; fp32 PSUM
     tiles [128,2,512] (2 banks, triple-buffered shared pool = 6 banks).
     exp on ScalarE (scale=ln2) directly from PSUM — measured faster than
     any VectorE-assisted scheme: DVE's PSUM port is ~4B/cyc/lane so
     staging 33M elements through SBUF loses more than ScalarE/PE PSUM
     contention costs. attnv accumulates [65,512] per head (row 64 =
     rowsum) in 2 PSUM banks, lagging PENDING_DEPTH=3 key tiles behind
     exp so the PE never stalls on a single ex tile.
     Block end: unnormalized out + den rows leave PSUM via 4 independent
     VectorE copies (ScalarE must not touch PSUM: catastrophic PE
     interference, measured 2.3x degradation).
  4. per head pair: one reciprocal_approx_fast over both heads' dens,
     gpsimd partition_broadcast, then in-place normalization multiplies
     (gpsimd + vector split) on the bf16 attn_outT slices.
  5. out = attn_outT.T @ w_out after the last pair normalizes (the
     out-proj matmuls share the sim PSUM pool).
  qk tiles for later head pairs are emitted between early blocks to fill
  TensorE slack under the exp-paced steady state.

A validated-but-unused fast path remains behind EXP_MODE flags: a custom
DVE op (EXP2_CORR_ANT) computing corrected Schraudolph exp2 from int32
bits at ~0.8% max rel err (bit-exact pieces verified on HW); per-j
VectorE work composes badly with the in-order engine queues, so the
all-ScalarE config wins end to end.
"""

import numpy as np

import concourse.bass as bass
import concourse.mybir as mybir
import concourse.tile as tile
from concourse import bacc
from concourse import dve_ops
from concourse.dve_spec import Spec, Src0, C0, C1, C2, One, Bin, lower
from concourse.dve_uop import AluOp, DveOpSpec
from concourse.masks import make_identity

FP32 = mybir.dt.float32
BF16 = mybir.dt.bfloat16
FP16 = mybir.dt.float16
I32 = mybir.dt.int32

B = 8
N = 2048
D = 512
H = 8
DH = 64
F3 = 3 * D
SCALE = DH**-0.5
LOG2E = 1.4426950408889634
LN2 = 0.6931471805599453
A0 = SCALE * LOG2E  # q-feature pre-scale: sim arrives in log2 units
P = 128
NT = N // P  # 16 token tiles
CT = D // P  # 4 contraction tiles over dim

# Schraudolph-with-correction constants (validated on HW, micro_dve*.py)
MASK_F = float(np.int32(0x007FFFFF).view(np.float32))
C_CORR = 0.2354900174785089  # f = 1 + C*(v-1)(2-v) = 1 + (+C)(v-1)(v-2)*(-1)
BITS_A = 8388608.0  # 2^23
BITS_B = 1065353216.0  # 127 << 23

# which key tiles the Vector engine exps (rest go to ScalarE)
DVE_JS = ()
# emit qk/out-proj matmuls between attention blocks (else serial phases)
INTERLEAVE = True
# "sbuf"  : DVE copies sim PSUM->SBUF fp16, ACT exps from SBUF (fast path —
#           ScalarE must NEVER read PSUM while the PE is streaming)
# "hybrid": tiles in SBUF_JS go DVE-copy(fp32)->ACT-sbuf-exp; rest ACT-psum
# "split" | "act" | "dve" | "none" (none = skip exp, for PE-path timing only)
EXP_MODE = "act"
SBUF_JS = (1, 3, 5, 7, 9, 11, 13, 15)
# how many j iterations attnv lags behind exp emission
PENDING_DEPTH = 3
# debug: skip sim matmuls ("nosim") or attnv matmuls ("noattnv")
SKIP = ""
# sim matmuls as 4 quadrant-tiled 64x64-key MMs (2 truly-concurrent pairs)
SIM_QUAD = True
# attnv as key-half pairs (A keys 0-63 + B keys 64-127 etc.)
ATTNV_PAIR = False
# engine for the 8 normalization multiplies: how many of 8 go to gpsimd
NORM_GPSIMD = 4
# exp key tiles in (pair, single) cadence: pairs land in a dedicated 4-bank
# PSUM tile so ScalarE reads 2048-wide (0.98 ns/elem vs 1.31 at 1024)
EXP_PAIRS = False  # pair cadence measured 430us vs 392-403 checkpoint (phase-1 bufs=1 serialization)
PAIR_STARTS = (0, 3, 6, 9, 12)  # j values starting a (j, j+1) pair
# put the unnA evacuation copy on ScalarE (frees VectorE, risks PSUM poison)
SCALAR_EVAC = False
# emit qk(0)/qk(4) before the v tiles and interleave v tiles 2-7 into block
# 0's exp-paced PE slack, so the exp stream starts ~27us earlier
V_EARLY = False


def _ref_exp2corr(in0, in1, s0, s1, imm2):
    b = in0.view(np.int32)
    mask = np.float32(s0).view(np.int32)
    oneb = np.float32(s1).view(np.int32)
    v = ((b & mask) | oneb).view(np.float32)
    return (in0 * (1.0 + imm2 * (v - 1.0) * (v - 2.0))).astype(np.float32)


def _register_exp2_op():
    name = "EXP2_CORR_ANT"
    if name in dve_ops._SUB_OPCODE_FOR_NAME:
        return next(op for op in dve_ops.OPS if op.name == name)
    _and = Bin(AluOp.BITWISE_AND, Src0, C0)
    _v = Bin(AluOp.BITWISE_OR, _and, C1)
    _p = _v - One
    _q = _p - One
    _m = _p * _q
    _f = One + _m * C2
    spec = Spec(body=Src0 * _f, reference=_ref_exp2corr)
    shas = {}
    for ver in ("v3", "v4"):
        s = DveOpSpec(name=name, opcode=0, uops=lower(spec, ver=ver), rd1_en=False)
        shas[ver] = s.sha(ver)
    op = dve_ops.DveOp(name, spec, subdim=False, uops_sha=shas)
    row = max(dve_ops._SUB_OPCODE_FOR_NAME.values()) + 1
    assert row < 0x20
    dve_ops.OPS.append(op)
    dve_ops._SUB_OPCODE_FOR_NAME[name] = row
    dve_ops.CUSTOM_DVE_SPECS[name] = spec
    return op


EXP2_OP = _register_exp2_op()


def _attention_body(tc: "tile.TileContext", repeat: int = 1):
    nc = tc.nc
    x = nc.dram_tensor("x", [N, D], FP32, kind="ExternalInput").ap()
    w_qkv = nc.dram_tensor("w_qkv", [D, F3], FP32, kind="ExternalInput").ap()
    w_out = nc.dram_tensor("w_out", [D, D], FP32, kind="ExternalInput").ap()
    out = nc.dram_tensor("out", [N, D], FP32, kind="ExternalOutput").ap()

    with (
        tc.tile_pool(name="const", bufs=1) as const,
        tc.tile_pool(name="persist", bufs=1) as persist,
        tc.tile_pool(name="wstage", bufs=2) as wstage,
    ):
        identity = const.tile([P, P], BF16)
        make_identity(nc, identity)
        ones32 = const.tile([P, 1], FP32)
        nc.vector.memset(ones32, 1.0)

        # q and k features transposed: rows = 1024 q/k features in 8 tiles
        qkT = persist.tile([P, 8, N], BF16)
        # v with tokens on partitions; per head 64 value cols + 1 ones col
        v_aug = persist.tile([P, NT, H * 65], BF16)
        nc.vector.tensor_copy(
            out=v_aug.rearrange("p j (h c) -> p j h c", c=65)[:, :, :, 64:65],
            in_=ones32.to_broadcast([P, NT, H, 1]),
        )
        # attention output (normalized in step 4), heads stacked in pairs:
        # tile t holds heads (2t, 2t+1) at rows 0-63 / 64-127
        attn_outT = persist.tile([P, CT, N], BF16)
        wout_sb = persist.tile([P, CT, D], BF16)
        for t in range(CT):
            ws = wstage.tile([P, F3], FP32, tag="ws")
            nc.sync.dma_start(out=ws[:, :D], in_=w_out[t * P : (t + 1) * P, :])
            nc.vector.tensor_copy(out=wout_sb[:, t, :], in_=ws[:, :D])
        # w_qkv cast to bf16 with q-feature output columns (0:512) pre-scaled
        wqkv_sb = persist.tile([P, CT, F3], BF16)
        for t in range(CT):
            ws = wstage.tile([P, F3], FP32, tag="ws")
            nc.sync.dma_start(out=ws, in_=w_qkv[t * P : (t + 1) * P, :])
            nc.vector.tensor_scalar_mul(
                out=wqkv_sb[:, t, 0:D], in0=ws[:, 0:D], scalar1=float(A0)
            )
            nc.vector.tensor_copy(out=wqkv_sb[:, t, D:F3], in_=ws[:, D:F3])

        consts = (identity, ones32, qkT, v_aug, attn_outT, wout_sb, wqkv_sb)
        for _ in range(repeat):
            _attention_once(tc, x, out, consts)


def _attention_once(tc: "tile.TileContext", x, out, consts):
    nc = tc.nc
    exp_f = mybir.ActivationFunctionType.Exp
    identity, ones32, qkT, v_aug, attn_outT, wout_sb, wqkv_sb = consts

    with (
        tc.tile_pool(name="proj", bufs=1) as proj_pool,
        tc.tile_pool(name="xstage", bufs=4) as xstage,
        tc.tile_pool(name="xbfst", bufs=4) as xbfst,
        # shared PSUM pool: sim tiles + qk/v/out-proj matmul tiles (2 banks
        # each, 3 bufs = 6 banks; pso pool below takes the last 2)
        tc.tile_pool(
            name="psmm", bufs=1 if EXP_PAIRS else 3, space="PSUM"
        ) as psmm,
        tc.tile_pool(name="pspair", bufs=1, space="PSUM") as pspair,
        tc.tile_pool(name="pso", bufs=1, space="PSUM") as psop,
        tc.tile_pool(name="ex", bufs=PENDING_DEPTH + 2) as expp,
        tc.tile_pool(name="simsb", bufs=3) as simsbp,
        tc.tile_pool(name="bits", bufs=2) as bitsp,
        tc.tile_pool(name="dpair", bufs=2) as dpairp,
        tc.tile_pool(name="rinv", bufs=1) as rinvp,
        tc.tile_pool(name="r0", bufs=1) as r0p,
        tc.tile_pool(name="db", bufs=1) as dbp,
        tc.tile_pool(name="outstage", bufs=2) as outstage,
    ):
        # ---- load x, cast bf16, transpose to xT [512, 2048] ----
        xT = proj_pool.tile([P, CT, N], BF16)
        for j in range(NT):
            xs = xstage.tile([P, D], FP32)
            nc.sync.dma_start(out=xs, in_=x[j * P : (j + 1) * P, :])
            xbf = xbfst.tile([P, D], BF16)
            nc.vector.tensor_copy(out=xbf, in_=xs)
            # transpose via shared psum ring (bf16 view of an fp32 tile)
            ps = psmm.tile([P, 2, 512], FP32, tag="mm")
            psb = ps.bitcast(BF16).rearrange("p a b -> p (a b)")
            for t in range(CT):
                nc.tensor.transpose(
                    psb[:, t * P : (t + 1) * P], xbf[:, t * P : (t + 1) * P], identity
                )
            nc.vector.tensor_copy(
                out=xT[:, :, j * P : (j + 1) * P],
                in_=psb[:, 0:512].rearrange("p (t q) -> p t q", q=P),
            )

        # ---- projections (qk feature tiles m; v tiles) ----
        def qk_tile(m, on_act=False):
            for nbp in range(2):
                ps = psmm.tile([P, 2, 512], FP32, tag="mm")
                for c in range(CT):
                    for q in range(2):
                        nc.tensor.matmul(
                            ps[:, q, :],
                            wqkv_sb[:, c, m * P : (m + 1) * P],
                            xT[:, c, (nbp * 2 + q) * 512 : (nbp * 2 + q + 1) * 512],
                            start=(c == 0),
                            stop=(c == CT - 1),
                        )
                src = ps.rearrange("p a b -> p (a b)")
                dst = qkT[:, m, nbp * 1024 : (nbp + 1) * 1024]
                if on_act:
                    nc.scalar.copy(out=dst, in_=src)
                else:
                    nc.vector.tensor_copy(out=dst, in_=src)

        def v_tile(jp):
            ps = psmm.tile([P, 2, 512], FP32, tag="mm")
            for c in range(CT):
                for q in range(2):
                    j = jp * 2 + q
                    nc.tensor.matmul(
                        ps[:, q, :],
                        xT[:, c, j * P : (j + 1) * P],
                        wqkv_sb[:, c, 2 * D : 3 * D],
                        start=(c == 0),
                        stop=(c == CT - 1),
                    )
            dst = v_aug[:, jp * 2 : jp * 2 + 2, :].rearrange(
                "p j (h c) -> p j h c", c=65
            )[:, :, :, 0:64]
            srcv = ps.rearrange("p q (h c) -> p q h c", c=64)
            nc.vector.tensor_copy(out=dst, in_=srcv)

        if V_EARLY:
            qk_tile(0)
            qk_tile(4)
            v_tile(0)
            v_tile(1)
        else:
            for jp in range(8):
                v_tile(jp)
            qk_tile(0)
            qk_tile(4)

        # ---- out-projection per token-pair tile (interleaved) ----
        def out_proj(jp):
            # alternate pools so the tail isn't serialized on one buffer
            if EXP_PAIRS and EXP_MODE == "act" and jp % 2 == 0:
                psbig = pspair.tile([P, 2, 2, 512], FP32, tag="pp")
                ps = psbig[:, 0, :, :]
            else:
                ps = psmm.tile([P, 2, 512], FP32, tag="mm")
            for t in range(CT):
                for q in range(2):
                    j = jp * 2 + q
                    nc.tensor.matmul(
                        ps[:, q, :],
                        attn_outT[:, t, j * P : (j + 1) * P],
                        wout_sb[:, t, :],
                        start=(t == 0),
                        stop=(t == CT - 1),
                    )
            os_ = outstage.tile([P, 2, D], FP32)
            nc.vector.tensor_copy(out=os_, in_=ps)
            nc.sync.dma_start(
                out=out[jp * 256 : (jp + 1) * 256, :].rearrange(
                    "(q p) d -> p q d", p=P
                ),
                in_=os_,
            )

        # ---- per-pair normalization over dpair[:, lo:hi] ----
        # pr 0..2 normalize their full 2048 q after the 4th block; pr 3
        # normalizes per 512-q chunk so out-proj can start immediately.
        def normalize(pr, dpair, lo, hi, sfx=""):
            w = hi - lo
            rv = rinvp.tile([33, w], FP32, tag="rv" + sfx)
            nc.vector.reciprocal_approx_fast(out=rv, in_=dpair[:, lo:hi])
            r0 = r0p.tile([1, w], FP32, tag="r0" + sfx)
            nc.vector.tensor_copy(out=r0, in_=rv[32:33, :])
            dbA = dbp.tile([P, w], FP32, tag="bA" + sfx)
            nc.gpsimd.partition_broadcast(dbA, rv[0:1, :])
            dbB = dbp.tile([P, w], FP32, tag="bB" + sfx)
            nc.gpsimd.partition_broadcast(dbB, r0)
            # in-place normalize attn_outT rows; split chunks across engines
            nchunks = max(1, w // 1024)
            csz = w // nchunks
            with nc.allow_low_precision("bf16 attn out"):
                for h, db, rowbase in ((0, dbA, 0), (1, dbB, 64)):
                    for c in range(nchunks):
                        sl = slice(lo + c * csz, lo + (c + 1) * csz)
                        dsl = slice(c * csz, (c + 1) * csz)
                        eng = (
                            nc.gpsimd
                            if (h * nchunks + c) % 4 < NORM_GPSIMD // 2
                            else nc.vector
                        )
                        eng.tensor_mul(
                            out=attn_outT[rowbase : rowbase + 64, pr, sl],
                            in0=attn_outT[rowbase : rowbase + 64, pr, sl],
                            in1=db[rowbase : rowbase + 64, dsl],
                        )

        # ---- attention blocks ----
        if INTERLEAVE:
            qkv_ = lambda m: qk_tile(m, on_act=False)
            extra = {
                0: [lambda: qkv_(1)],
                1: [lambda: qkv_(5)],
                4: [lambda: qkv_(2)],
                5: [lambda: qkv_(6)],
                8: [lambda: qkv_(3)],
                9: [lambda: qkv_(7)],
                # out-proj waits for normalize(pr=3) after block 15; a
                # per-qc pr=3 normalization + earlier out-proj interleave
                # measured 45us WORSE (norm ops gate out-proj, PE stalls)
                15: [lambda jp=jp: out_proj(jp) for jp in range(8)],
            }
        else:
            for m in (1, 5, 2, 6, 3, 7):
                qk_tile(m)
            extra = {15: [lambda jp=jp: out_proj(jp) for jp in range(8)]}

        ex_const = None
        if EXP_MODE == "none":
            ex_const = proj_pool.tile([P, 2, 512], BF16)
            nc.vector.memset(ex_const, 0.001)

        for pr in range(H // 2):
            hA, hB = 2 * pr, 2 * pr + 1
            qt, kt = pr, 4 + pr
            # unnormalized denominators for this pair: row 0 = A, row 32 = B
            dpair = dpairp.tile([33, N], FP32, tag="d")
            for qc in range(4):
                bi = pr * 4 + qc
                qsl = slice(qc * 512, (qc + 1) * 512)
                psoA = psop.tile([65, 512], FP32, tag="A")
                psoB = psop.tile([65, 512], FP32, tag="B")

                def attnv(ex_t, pj):
                    if ATTNV_PAIR and SKIP != "noattnv":
                        first, last = pj == 0, pj == NT - 1
                        # row-tiled pairs: (A keys 0-63 | B keys 64-127),
                        # then (B keys 0-63 | A keys 64-127)
                        for pso, hh, kh, idx in (
                            (psoA, hA, 0, 0),
                            (psoB, hB, 1, 0),
                            (psoB, hB, 0, 1),
                            (psoA, hA, 1, 1),
                        ):
                            rows = slice(64 * kh, 64 * kh + 64)
                            hcol = 0 if pso is psoA else 1
                            nc.tensor.matmul(
                                pso,
                                v_aug[rows, pj, hh * 65 : (hh + 1) * 65],
                                ex_t[rows, hcol, :],
                                start=(first and idx == 0),
                                stop=(last and idx == 1),
                                tile_position=(64 * kh, 0),
                            )
                        return
                    if SKIP == "noattnv":
                        if pj == NT - 1:
                            nc.tensor.matmul(
                                psoA, v_aug[:, 0, 0:65], ex_t[:, 0, :],
                                start=True, stop=True,
                            )
                            nc.tensor.matmul(
                                psoB, v_aug[:, 0, 0:65], ex_t[:, 1, :],
                                start=True, stop=True,
                            )
                        return
                    nc.tensor.matmul(
                        psoA,
                        v_aug[:, pj, hA * 65 : (hA + 1) * 65],
                        ex_t[:, 0, :],
                        start=(pj == 0),
                        stop=(pj == NT - 1),
                    )
                    nc.tensor.matmul(
                        psoB,
                        v_aug[:, pj, hB * 65 : (hB + 1) * 65],
                        ex_t[:, 1, :],
                        start=(pj == 0),
                        stop=(pj == NT - 1),
                    )

                def sim_mms(j, dst):
                    # four 64-feat x 64-key MMs in disjoint array
                    # quadrants; pairs stream concurrently.
                    # (head h on array rows 64h.., key half kh on
                    #  array cols 64kh -> psum partitions 64kh..)
                    for h, kh in ((0, 0), (1, 1), (0, 1), (1, 0)):
                        rows = slice(64 * h, 64 * h + 64)
                        ksl = slice(j * P + 64 * kh, j * P + 64 * kh + 64)
                        nc.tensor.matmul(
                            dst[64 * kh : 64 * kh + 64, h, :],
                            qkT[rows, kt, ksl],
                            qkT[rows, qt, qsl],
                            start=True,
                            stop=True,
                            tile_position=(64 * h, 64 * kh),
                        )

                use_pairs = (
                    EXP_PAIRS and EXP_MODE == "act" and SKIP == "" and SIM_QUAD
                )
                pending = []
                j = 0
                while j < NT:
                    if use_pairs and j in PAIR_STARTS:
                        # 2048-wide exp: two key tiles in one 4-bank tile
                        pt = pspair.tile([P, 2, 2, 512], FP32, tag="pp")
                        sim_mms(j, pt[:, 0, :, :])
                        sim_mms(j + 1, pt[:, 1, :, :])
                        ex2 = expp.tile([P, 2, 2, 512], BF16, tag="ex2")
                        nc.scalar.activation(
                            out=ex2.rearrange("p a b c -> p (a b c)"),
                            in_=pt.rearrange("p a b c -> p (a b c)"),
                            func=exp_f,
                            scale=LN2,
                        )
                        pending.append((ex2[:, 0, :, :], j))
                        pending.append((ex2[:, 1, :, :], j + 1))
                        while len(pending) > PENDING_DEPTH:
                            attnv(*pending.pop(0))
                        j += 2
                        continue
                    simt = None
                    if SKIP != "nosim":
                        simt = psmm.tile([P, 2, 512], FP32, tag="mm")
                        if SIM_QUAD:
                            sim_mms(j, simt)
                        else:
                            nc.tensor.matmul(
                                simt[:, 0, :],
                                qkT[0:64, kt, j * P : (j + 1) * P],
                                qkT[0:64, qt, qsl],
                                start=True,
                                stop=True,
                                tile_position=(0, 0),
                            )
                            nc.tensor.matmul(
                                simt[:, 1, :],
                                qkT[64:128, kt, j * P : (j + 1) * P],
                                qkT[64:128, qt, qsl],
                                start=True,
                                stop=True,
                                tile_position=(64, 0),
                            )
                    elif EXP_MODE != "none":
                        simt = psmm.tile([P, 2, 512], FP32, tag="mm")
                        nc.vector.memset(simt, 0.5)
                    use_dve = (EXP_MODE == "split" and j in DVE_JS) or (
                        EXP_MODE == "dve"
                    )
                    if EXP_MODE == "none":
                        ex_t = ex_const
                    elif EXP_MODE in ("sbuf", "hybrid"):
                        ex_t = expp.tile([P, 2, 512], BF16, tag="ex")
                        if EXP_MODE == "sbuf" or j in SBUF_JS:
                            sim_sb = simsbp.tile([P, 2, 512], FP32, tag="ss")
                            nc.vector.tensor_copy(out=sim_sb, in_=simt)
                            src_ap = sim_sb
                        else:
                            src_ap = simt
                        nc.scalar.activation(
                            out=ex_t.rearrange("p a b -> p (a b)"),
                            in_=src_ap.rearrange("p a b -> p (a b)"),
                            func=exp_f,
                            scale=LN2,
                        )
                    elif use_dve:
                        ex_t = expp.tile([P, 2, 512], BF16, tag="ex")
                        bits = bitsp.tile([P, 2, 512], I32, tag="bits")
                        nc.vector.tensor_scalar(
                            out=bits,
                            in0=simt,
                            scalar1=BITS_A,
                            scalar2=BITS_B,
                            op0=mybir.AluOpType.mult,
                            op1=mybir.AluOpType.add,
                        )
                        nc.vector._custom_dve(
                            EXP2_OP,
                            out=ex_t,
                            in0=bits.bitcast(FP32),
                            s0=MASK_F,
                            s1=1.0,
                            imm2=C_CORR,
                        )
                    else:
                        ex_t = expp.tile([P, 2, 512], BF16, tag="ex")
                        nc.scalar.activation(
                            out=ex_t.rearrange("p a b -> p (a b)"),
                            in_=simt.rearrange("p a b -> p (a b)"),
                            func=exp_f,
                            scale=LN2,
                        )
                    pending.append((ex_t, j))
                    if V_EARLY and bi == 0 and j in (2, 4, 6, 8, 10, 12):
                        v_tile(j // 2 + 1)
                    if len(pending) > PENDING_DEPTH:
                        attnv(*pending.pop(0))
                    j += 1
                for p_ in pending:
                    attnv(*p_)

                # evacuate: 4 independent copies, pso banks free ASAP
                # (VectorE only — ScalarE reading PSUM stalls the PE)
                with nc.allow_low_precision("bf16 attn out"):
                    if SCALAR_EVAC:
                        nc.scalar.copy(
                            out=attn_outT[0:64, pr, qsl], in_=psoA[0:64, :]
                        )
                    else:
                        nc.vector.tensor_copy(
                            out=attn_outT[0:64, pr, qsl], in_=psoA[0:64, :]
                        )
                    nc.vector.tensor_copy(
                        out=attn_outT[64:128, pr, qsl], in_=psoB[0:64, :]
                    )
                nc.vector.tensor_copy(
                    out=dpair[0:1, qsl], in_=psoA[64:65, :]
                )
                nc.vector.tensor_copy(
                    out=dpair[32:33, qsl], in_=psoB[64:65, :]
                )

                if EXP_MODE != "none" and bi % 4 == 3:
                    normalize(pr, dpair, 0, N)
                for thunk in extra.get(bi, ()):
                    thunk()


_CACHE: dict = {}


def build_nc(repeat: int = 1) -> "bass.Bass":
    key = (
        "nc", repeat, EXP_MODE, DVE_JS, INTERLEAVE, NORM_GPSIMD,
        PENDING_DEPTH, SKIP, SBUF_JS, SIM_QUAD, ATTNV_PAIR, SCALAR_EVAC,
        EXP_PAIRS, PAIR_STARTS, V_EARLY,
    )
    if key not in _CACHE:
        nc = bacc.Bacc("TRN2", target_bir_lowering=False, debug=False)
        with tile.TileContext(nc) as tc:
            _attention_body(tc, repeat=repeat)
        nc.compile()
        _CACHE[key] = nc
    return _CACHE[key]


def kernel(x: np.ndarray, w_qkv: np.ndarray, w_out: np.ndarray) -> np.ndarray:
    from concourse.bass_utils import run_bass_kernel_spmd

    nc = build_nc()
    x = np.ascontiguousarray(np.asarray(x, dtype=np.float32))
    w_qkv = np.ascontiguousarray(np.asarray(w_qkv, dtype=np.float32))
    w_out = np.ascontiguousarray(np.asarray(w_out, dtype=np.float32))
    in_maps = [
        {"x": x[i], "w_qkv": w_qkv, "w_out": w_out} for i in range(B)
    ]
    res = run_bass_kernel_spmd(nc, in_maps, core_ids=list(range(B)))
    return np.stack([r["out"] for r in res.results], axis=0)
